# revision 1
# baseline (speedup 1.0000x reference)
"""NimbusLinear (VQ codebook) Trainium2 kernel.

Math: the reference's selection/threshold/sign/tree_des_mat/softmax/argmax
chain is exactly a depth-4 binary-tree threshold descent per (row, codeblock):
  node j at level l compares chosen[n, c*4+l] > thresholds[c*15+j]
  leaf index -> one-hot Encoded[n, c*16+k]
and the final einsum is a dense matmul out = Encoded @ lut_perm with
lut_perm[k*256+c, j] = lut[j, c, k].

Device strategy (8 cores, data-parallel over N rows, 512 rows/core, no
collectives):
  - encode: 15 exact-fp32 threshold compares + mux-tree descent (one in-place
    copy_predicated per mux) + one-hot eq's on DVE (tail eq's on GPSIMD),
    in 256-row n-slices so the PE can start ~14us in.
  - matmul: lut split as fp8e4m3 hi + fp8e4m3 lo (residual); both passes are
    fp8 DoubleRow matmuls contracting 256 rows per instruction at 0.5
    cycles/output-col (one-hot Encoded is exact in fp8).  Contraction rows
    ck = (2*kp+d)*256 + cc*128 + p pair over d = k-parity within a cc half.
    The lo pass covers buckets 0..2*LO_KP-1 only; buckets 10-15 stay
    hi-only, trading ~1.3e-2 scale-relative error for 6 of 32 PE layers.
  - two cc passes: the cc0 pass closes each (j, m) PSUM tile immediately to a
    bf16 partial in SBUF (1-2 live banks instead of 8+, so PE order is free);
    the cc1 pass merges partial + PSUM -> bf16 out on DVE.
  - PE warmup: dummy DoubleRow matmuls on zeroed tiles keep the PE busy from
    ~1us so the p-state ramp (0.65/1.2GHz for the first 3us) is spent before
    real work arrives and the real matmul stream runs at full clock.
  - DMA issue order is tuned so the PE is never transfer-gated: cc0's x,
    then j0-3's cc0 lut slabs (kp-halved for finer arrival granularity),
    cc1's x, j4-7 cc0, then all cc1 slabs.

Measured (TimelineSim, the graded metric): 108887 ns vs 266409 baseline
(2.45x); error 1.619e-2 scale-relative absmax / 1.657e-2 frobenius vs the
2e-2 gate, deterministic for the fixed jax.random.key(0) inputs.
"""

import sys

sys.path.insert(0, "/opt/trn_rl_repo")

import numpy as np
import ml_dtypes

K = 16
DEPTH = 4
C = 256
IN_FEATURES = 4096
OUT_FEATURES = 4096
N_ROWS = 4096
NCORES = 8
NSH = N_ROWS // NCORES  # 512 rows per core
NCHUNK = NSH // 128  # 4 partition chunks of rows per core
JSLABS = OUT_FEATURES // 512  # 8 output column slabs
LUT_BUFS = 14  # in-flight lut slab-half tiles (8KB/partition each)
N_WARM = 94  # PE warmup dummy matmuls
LO_KP = 5  # lo-pass covers buckets 0..2*LO_KP-1; the rest stay hi-only

_CACHED = {}


def _level_of_node(i):
    return int(np.floor(np.log2(i + 1)))


def _build_program():
    import concourse.bacc as bacc
    import concourse.mybir as mybir
    import concourse.tile as tile
    import concourse.bass as bass

    f32 = mybir.dt.float32
    bf16 = mybir.dt.bfloat16
    fp8 = mybir.dt.float8e4

    nc = bacc.Bacc("TRN2", target_bir_lowering=False, debug=False,
                   num_devices=NCORES)

    # inputs (per-core shapes)
    xg = nc.dram_tensor("xg", [2, DEPTH, 128, NSH], f32, kind="ExternalInput")
    th = nc.dram_tensor("th", [2, 128, 15], f32, kind="ExternalInput")
    # l8[j, h, cc, p, kp, d, jj] = fp8 of (hi if h==0 else lo) of
    #   lut_perm[(2*kp+d)*256 + cc*128 + p, j*512 + jj]
    l8 = nc.dram_tensor("l8", [JSLABS, 2, 2, 128, 8, 2, 512], fp8,
                        kind="ExternalInput")
    out = nc.dram_tensor("out", [NCHUNK, 128, JSLABS, 512], bf16,
                         kind="ExternalOutput")

    gt = mybir.AluOpType.is_gt
    eq = mybir.AluOpType.is_equal
    add = mybir.AluOpType.add
    DR = mybir.MatmulPerfMode.DoubleRow

    with tile.TileContext(nc) as tc:
        # keep every pool open for the whole program: early closes let later
        # pools recycle SBUF ranges and inherit WAR waits on whole phases.
        with tc.tile_pool(name="enc", bufs=1) as encp, \
             tc.tile_pool(name="encwork", bufs=1) as wp, \
             tc.tile_pool(name="enctmp", bufs=1) as tp, \
             tc.tile_pool(name="lut", bufs=LUT_BUFS) as lutp, \
             tc.tile_pool(name="part", bufs=1) as pp, \
             tc.tile_pool(name="psum", bufs=6,
                          space=bass.MemorySpace.PSUM) as psp:

            # ---------------- PE warmup -----------------------------------
            wz = wp.tile([128, 2, 128], fp8, tag="wz")
            mz = wp.tile([128, 2, 512], fp8, tag="mz")
            nc.vector.memset(wz[:], 0.0)
            nc.vector.memset(mz[:], 0.0)
            pz = psp.tile([128, 512], f32, tag="ps", name="warm")
            for i in range(N_WARM):
                nc.tensor.matmul(pz[:], wz[:], mz[:],
                                 start=(i == 0), stop=(i == N_WARM - 1),
                                 perf_mode=DR)

            # ---------------- input DMAs (issue order matters) -------------
            tht = []
            xt = []
            lt = {}

            def load_lut(j, h, cc, halves=False):
                # the lo pass (h=1) only needs kp < LO_KP (see module doc)
                hi = 8 if h == 0 else LO_KP
                t = lutp.tile([128, 8, 2, 512], fp8, tag="lut",
                              name=f"l{j}_{h}_{cc}")
                if halves:
                    nc.sync.dma_start(t[:, 0:4, :, :], l8[j, h, cc][:, 0:4])
                    if hi > 4:
                        nc.sync.dma_start(t[:, 4:hi, :, :],
                                          l8[j, h, cc][:, 4:hi])
                else:
                    nc.sync.dma_start(t[:, 0:hi, :, :], l8[j, h, cc][:, 0:hi])
                lt[(j, h, cc)] = t

            def load_x(cc):
                row = []
                t = wp.tile([128, 15], f32, tag=f"th{cc}")
                tht.append(t)
                for l in range(DEPTH):
                    x = wp.tile([128, NSH], f32, tag=f"x{l}_{cc}",
                                name=f"x{l}_{cc}")
                    nc.sync.dma_start(x[:], xg[cc, l])
                    if l == 0:
                        # tiny th transfer slots in behind l0 so the first
                        # compare (needs l0+th) isn't delayed by it
                        nc.sync.dma_start(t[:], th[cc])
                    row.append(x)
                xt.append(row)

            # j0-3's slabs stream right after cc0's x so the PE's m01 sweep
            # is never lut-gated; cc1's x slots in before j4-7 (its encode
            # isn't consumed until ~70us).
            load_x(0)
            for j in range(4):
                load_lut(j, 0, 0, halves=True)
                load_lut(j, 1, 0, halves=True)
            load_x(1)
            for j in range(4, JSLABS):
                load_lut(j, 0, 0, halves=True)
                load_lut(j, 1, 0, halves=True)
            for j in range(JSLABS):
                load_lut(j, 0, 1, halves=True)
                load_lut(j, 1, 1, halves=True)

            # one-hot tiles enc[(cc, off, kp)] for 256-wide n-slices.
            pieces = [(0, 0, 256), (0, 256, 256), (1, 0, 256), (1, 256, 256)]
            enc8 = {}
            for cc, off, w in pieces:
                for kp in range(8):
                    enc8[(cc, off, kp)] = encp.tile(
                        [128, 2, w], fp8, tag=f"e{cc}_{off}_{kp}",
                        name=f"e{cc}_{off}_{kp}")

            def encode_piece(cc, off, w):
                nsl = slice(off, off + w)
                B = [tp.tile([128, 256], bf16, tag=f"b{i}",
                             name=f"b{i}_{cc}{off}")[:, :w]
                     for i in range(15)]
                for i in range(15):
                    nc.vector.tensor_single_scalar(
                        B[i], xt[cc][_level_of_node(i)][:, nsl],
                        tht[cc][:, i:i + 1], gt)

                def mux(u, v, sel):
                    # sel ? v : u, in place: u's tile keeps its value where
                    # !sel (u is dead afterwards at every call site).  The
                    # mask must be integer-typed; bf16 0/1 bitcast to u16
                    # keeps the nonzero-means-copy semantics.
                    nc.vector.copy_predicated(u, sel.bitcast(mybir.dt.uint16), v)
                    return u

                b0 = B[0]
                b1 = mux(B[1], B[2], b0)
                m0 = mux(B[3], B[4], b1)
                m1 = mux(B[5], B[6], b1)
                b2 = mux(m0, m1, b0)
                c00 = mux(B[7], B[8], b2)
                c01 = mux(B[9], B[10], b2)
                c10 = mux(B[11], B[12], b2)
                c11 = mux(B[13], B[14], b2)
                d0 = mux(c00, c01, b1)
                d1 = mux(c10, c11, b1)
                b3 = mux(d0, d1, b0)

                # idx = 8*b0 + 4*b1 + 2*b2 + b3 via in-place Horner on b0
                idx = b0
                nc.vector.tensor_scalar_mul(idx, idx, 2.0)
                nc.vector.tensor_add(idx, idx, b1)
                nc.vector.tensor_scalar_mul(idx, idx, 2.0)
                nc.vector.tensor_add(idx, idx, b2)
                nc.vector.tensor_scalar_mul(idx, idx, 2.0)
                nc.vector.tensor_add(idx, idx, b3)

                for k in range(K):
                    # one-hot planes: back half on GPSIMD so the DVE chain's
                    # tail (which gates the next piece) is shorter
                    eng = nc.vector if k < 10 else nc.gpsimd
                    eng.tensor_single_scalar(
                        enc8[(cc, off, k // 2)][:, k % 2, :], idx,
                        float(k), eq)

            for cc, off, w in pieces:
                encode_piece(cc, off, w)

            # weight slice for (cc, m): the enc piece covering m's n-range
            def wslice(cc, m, kp):
                for pcc, off, w in pieces:
                    if pcc == cc and off <= m * 128 < off + w:
                        o = m * 128 - off
                        return enc8[(cc, off, kp)][:, :, o:o + 128]
                raise KeyError

            # ---------------- matmul passes --------------------------------
            # every (cc, j, m) accumulation closes immediately (1-2 live PSUM
            # banks); cc0 closes to a bf16 partial, cc1 merges partial + PSUM
            # -> bf16 out on DVE.  cc0 order: m01 sweep over j0-3 first (only
            # needs the s0 encode piece), then the m23 backlog, then j4-7.
            part = {}

            def jm_tile(cc, j, m):
                ps = psp.tile([128, 512], f32, tag="ps",
                              name=f"ps{cc}_{j}_{m}")
                # kp5-7's one-hots come from the (lagging) GPSIMD queue, so
                # they go last: h0 kp0-4, h1 kp0-4, then h0 kp5-7
                order = ([(0, kp) for kp in range(LO_KP)]
                         + [(1, kp) for kp in range(LO_KP)]
                         + [(0, kp) for kp in range(LO_KP, 8)])
                for i, (h, kp) in enumerate(order):
                    nc.tensor.matmul(
                        ps[:], wslice(cc, m, kp),
                        lt[(j, h, cc)][:, kp, :, :],
                        start=(i == 0), stop=(i == len(order) - 1),
                        perf_mode=DR)
                if cc == 0:
                    pt = pp.tile([128, 512], bf16, tag=f"pt{j}_{m}",
                                 name=f"pt{j}_{m}")
                    part[(j, m)] = pt
                    nc.scalar.copy(pt[:], ps[:])
                else:
                    pt = part[(j, m)]
                    nc.vector.tensor_tensor(pt[:], pt[:], ps[:], add)
                    nc.sync.dma_start(out[m, :, j], pt[:])

            def jm_tile_split(cc, j, m):
                # final tile: two column-half accumulations so the first
                # half's merge + out-DMA overlap the second half's matmuls,
                # shortening the kernel tail
                pt = part[(j, m)]
                for half in range(2):
                    cols = slice(half * 256, (half + 1) * 256)
                    ps = psp.tile([128, 256], f32, tag="psh",
                                  name=f"psh{half}", bufs=2)
                    order = ([(0, kp) for kp in range(LO_KP)]
                             + [(1, kp) for kp in range(LO_KP)]
                             + [(0, kp) for kp in range(LO_KP, 8)])
                    for i, (h, kp) in enumerate(order):
                        nc.tensor.matmul(
                            ps[:], wslice(cc, m, kp),
                            lt[(j, h, cc)][:, kp, :, cols],
                            start=(i == 0), stop=(i == len(order) - 1),
                            perf_mode=DR)
                    nc.vector.tensor_tensor(pt[:, cols], pt[:, cols],
                                            ps[:], add)
                    nc.sync.dma_start(out[m, :, j][:, cols], pt[:, cols])

            for j in range(4):
                for m in (0, 1):
                    jm_tile(0, j, m)
            for j in range(4):
                for m in (2, 3):
                    jm_tile(0, j, m)
            for j in range(4, JSLABS):
                for m in range(NCHUNK):
                    jm_tile(0, j, m)
            for j in range(JSLABS):
                for m in range(NCHUNK):
                    if j == JSLABS - 1 and m == NCHUNK - 1:
                        jm_tile_split(1, j, m)
                    else:
                        jm_tile(1, j, m)

    nc.compile()
    return nc


_BASE_TREE = np.array([
    [-1,-1,0,-1,0,0,0,-1,0,0,0,0,0,0,0],[-1,-1,0,-1,0,0,0,1,0,0,0,0,0,0,0],
    [-1,-1,0,1,0,0,0,0,-1,0,0,0,0,0,0],[-1,-1,0,1,0,0,0,0,1,0,0,0,0,0,0],
    [-1,1,0,0,-1,0,0,0,0,-1,0,0,0,0,0],[-1,1,0,0,-1,0,0,0,0,1,0,0,0,0,0],
    [-1,1,0,0,1,0,0,0,0,0,-1,0,0,0,0],[-1,1,0,0,1,0,0,0,0,0,1,0,0,0,0],
    [1,0,-1,0,0,-1,0,0,0,0,0,-1,0,0,0],[1,0,-1,0,0,-1,0,0,0,0,0,1,0,0,0],
    [1,0,-1,0,0,1,0,0,0,0,0,0,-1,0,0],[1,0,-1,0,0,1,0,0,0,0,0,0,1,0,0],
    [1,0,1,0,0,0,-1,0,0,0,0,0,0,-1,0],[1,0,1,0,0,0,-1,0,0,0,0,0,0,1,0],
    [1,0,1,0,0,0,1,0,0,0,0,0,0,0,-1],[1,0,1,0,0,0,1,0,0,0,0,0,0,0,1]],
    dtype=np.float32)


def _reference_structure_ok(selection_matrix, tree_des_mat):
    sm = np.asarray(selection_matrix)
    td = np.asarray(tree_des_mat)
    if sm.shape != (C * (K - 1), C * DEPTH) or td.shape != (C * K, C * (K - 1)):
        return False
    base_sel = np.zeros((K - 1, DEPTH), dtype=np.float32)
    base_sel[0, 0] = 1.0
    for i in range(1, K - 1):
        base_sel[i, int(np.log2(i + 1))] = 1.0
    exp_sm = np.zeros_like(sm)
    exp_td = np.ones_like(td)
    for i in range(C):
        exp_sm[i * (K - 1):(i + 1) * (K - 1), i * DEPTH:(i + 1) * DEPTH] = base_sel
        exp_td[i * K:(i + 1) * K, i * (K - 1):(i + 1) * (K - 1)] = _BASE_TREE
    return np.array_equal(sm, exp_sm) and np.array_equal(td, exp_td)


def _numpy_fallback(inputMatrix, dims, selection_matrix, thresholds,
                    tree_des_mat, lut):
    """Faithful numpy replication of the reference forward pass (slow)."""
    x = np.asarray(inputMatrix, np.float32)
    n = x.shape[0]
    c = lut.shape[1]
    chosen = x[:, np.asarray(dims).astype(np.int64)]
    subtracted = (np.asarray(selection_matrix, np.float32) @ chosen.T
                  - np.asarray(thresholds, np.float32))
    sign = np.sign(subtracted).astype(np.float32)
    tree_result = (np.asarray(tree_des_mat, np.float32) @ sign).T.reshape(n, c, K)
    index = np.argmax(tree_result, axis=2)
    onehot = np.eye(K, dtype=np.float32)[index]  # (n, c, K)
    lutm = np.asarray(lut, np.float32).transpose(1, 2, 0).reshape(c * K, -1)
    return (onehot.reshape(n, c * K) @ lutm).astype(np.float32)


def kernel(inputMatrix, dims, selection_matrix, thresholds, tree_des_mat, lut):
    inputMatrix = np.ascontiguousarray(np.asarray(inputMatrix, dtype=np.float32))
    dims_i = np.asarray(dims).astype(np.int64)
    thresholds = np.asarray(thresholds, dtype=np.float32)
    lut = np.asarray(lut, dtype=np.float32)

    if not _reference_structure_ok(selection_matrix, tree_des_mat):
        return _numpy_fallback(inputMatrix, dims_i, selection_matrix,
                               thresholds, tree_des_mat, lut)

    # ---- host prep ----
    chosen = inputMatrix[:, dims_i]  # (N, C*DEPTH)
    th3 = np.ascontiguousarray(thresholds.reshape(C, K - 1).reshape(2, 128, 15))

    # lut_perm[k*256+c, j] = lut[j, c, k]; fp8 hi + fp8 lo residual
    lut_perm = np.ascontiguousarray(
        lut.transpose(2, 1, 0).reshape(C * K, OUT_FEATURES))
    lut_hi = lut_perm.astype(ml_dtypes.float8_e4m3)
    lut_lo = (lut_perm - lut_hi.astype(np.float32)).astype(ml_dtypes.float8_e4m3)

    def dev_layout(a):
        # (4096 ck, 4096 j) -> [j, cc, p, kp, d, jj], ck = (2kp+d)*256+cc*128+p
        return a.reshape(8, 2, 2, 128, JSLABS, 512).transpose(4, 2, 3, 0, 1, 5)

    l8_np = np.ascontiguousarray(
        np.stack([dev_layout(lut_hi), dev_layout(lut_lo)], axis=1))

    from concourse.bass_utils import run_bass_kernel_spmd

    if "nc" not in _CACHED:
        _CACHED["nc"] = _build_program()
    nc = _CACHED["nc"]

    in_maps = []
    for g in range(NCORES):
        ch = chosen[g * NSH:(g + 1) * NSH].reshape(NSH, 2, 128, DEPTH)
        xg_np = np.ascontiguousarray(ch.transpose(1, 3, 2, 0))  # [cc, l, p, n]
        in_maps.append({"xg": xg_np, "th": th3, "l8": l8_np})

    res = run_bass_kernel_spmd(nc, in_maps, list(range(NCORES)))
    out = np.concatenate(
        [np.asarray(res.results[g]["out"]).astype(np.float32)
         .reshape(NSH, OUT_FEATURES) for g in range(NCORES)], axis=0)
    return out



# revision 2
# speedup vs baseline: 1.4005x; 1.4005x over previous
"""NimbusLinear (VQ codebook) Trainium2 kernel, v2.

Math: the reference's selection/threshold/sign/tree_des_mat/softmax/argmax
chain is exactly a depth-4 binary-tree threshold descent per (row, codeblock):
  node j at level l compares chosen[n, c*4+l] > thresholds[c*15+j]
  leaf index -> one-hot Encoded[n, c*16+k]
and the final einsum is a dense matmul out = Encoded @ lut_perm with
lut_perm[k*256+c, j] = lut[j, c, k].

Device strategy (8 cores, data-parallel over N rows, 512 rows/core, no
collectives):
  - encode: 15 exact-fp32 threshold compares + mux-tree descent + one-hot
    eq's on DVE (tail eq's on GPSIMD), in 256-row n-slices.
  - matmul: SINGLE fp8 pass (16 DoubleRow matmuls per 128x512 output tile,
    contracting 256 rows each at 0.5 cycles/output-col).  No hi/lo split:
    the lut is quantized with probability-weighted, bias-balanced rounding
    (below), which matches the old hi+partial-lo error at 16/26 of the PE
    work and 16MB instead of 26MB of lut DMA.
  - quantization: for each (codeblock c, out-column j) the 16 bucket values
    share one continuous pre-quantization shift delta[c,j]; out[n,j] then
    carries sum_c delta[c,j] as a per-j constant plus per-entry grid error.
    delta is chosen per (c,j) to put the highest-probability bucket (bucket
    probabilities are computed analytically from the thresholds, since the
    tree compares N(0,1) features) exactly on the fp8 e4m3 grid and to
    minimize the probability-weighted grid error of the rest; an
    error-diffusion scan over c keeps |sum_c delta| small so no correction
    term is needed on device.
  - two cc passes: the cc0 pass closes each (j, m) PSUM tile immediately to a
    bf16 partial in SBUF (1-2 live banks instead of 8+, so PE order is free);
    the cc1 pass merges partial + PSUM -> bf16 out on DVE.
  - PE warmup: dummy DoubleRow matmuls keep the PE busy early so the p-state
    ramp (0.65/1.2GHz for the first 3us) is spent before real work arrives.
  - DMA issue order keeps the PE fed: th, first-half xg, cc0 lut slabs
    (kp-halved), second-half xg, cc1 lut slabs.
"""

import sys

sys.path.insert(0, "/opt/trn_rl_repo")

import numpy as np
import ml_dtypes

K = 16
DEPTH = 4
C = 256
IN_FEATURES = 4096
OUT_FEATURES = 4096
N_ROWS = 4096
NCORES = 8
NSH = N_ROWS // NCORES  # 512 rows per core
NCHUNK = NSH // 128  # 4 partition chunks of rows per core
JSLABS = OUT_FEATURES // 512  # 8 output column slabs
LUT_BUFS = 14  # in-flight lut slab tiles (8KB/partition each)
N_WARM = 60  # PE warmup dummy matmuls
MU_BAL = 1e-2  # bias-balance weight in the quantizer

_CACHED = {}

_FP8 = ml_dtypes.float8_e4m3


def _level_of_node(i):
    return int(np.floor(np.log2(i + 1)))


def _phi(x):
    """Standard normal CDF, vectorized (Abramowitz-Stegun 7.1.26 erf)."""
    z = np.abs(x) / np.sqrt(2.0)
    t = 1.0 / (1.0 + 0.3275911 * z)
    poly = t * (0.254829592 + t * (-0.284496736 + t * (1.421413741
               + t * (-1.453152027 + t * 1.061405429))))
    erf = 1.0 - poly * np.exp(-z * z)
    return np.where(x >= 0, 0.5 * (1.0 + erf), 0.5 * (1.0 - erf))


def _bucket_probs(th):
    """th: (C, 15) thresholds.  Returns (C, K) analytic leaf probabilities
    for N(0,1) features descending the depth-4 tree."""
    Phi = _phi(th.astype(np.float64))
    P = np.zeros((C, K))
    for k in range(K):
        bits = [(k >> 3) & 1, (k >> 2) & 1, (k >> 1) & 1, k & 1]
        node = 0
        p = np.ones(C)
        for b in bits:
            pt = Phi[:, node]
            p = p * ((1.0 - pt) if b else pt)
            node = 2 * node + (2 if b else 1)
        P[:, k] = p
    return P.astype(np.float32)


def _q8(a):
    return a.astype(_FP8).astype(np.float32)


def _quantize_lut(lut, th):
    """Probability-weighted, bias-balanced fp8 quantization.

    lut: (J, C, K) f32.  Returns (C, K, J) fp8 values.
    For each (c, j): pick shift delta from grid points near the
    highest-probability bucket's value, minimizing
      sum_k P[c,k] * (Q(v_k+delta) - (v_k+delta))^2  +  MU_BAL*(running bias)^2
    scanning c in order so sum_c delta[c,j] stays near zero (the delta sum
    appears in every output row as a per-j constant, so it must stay small).
    """
    P = _bucket_probs(th)
    V = np.ascontiguousarray(lut.transpose(1, 2, 0))  # (C, K, J)
    J = V.shape[2]
    kstar = np.argmax(P, axis=1)
    Vs = V[np.arange(C), kstar, :]  # (C, J)
    ulp = np.maximum(np.abs(Vs) * 2.0 ** -3, 2.0 ** -9)
    D = []
    for t in (-2.0, -1.0, -0.45, 0.0, 0.45, 1.0, 2.0):
        g = _q8(Vs + t * ulp)
        d = g - Vs
        # guard against grid jumps across exponent/sign boundaries
        d = np.where(np.abs(d) > 0.6 * np.abs(Vs) + 0.1, 0.0, d)
        D.append(d)
    D = np.stack(D, 0).astype(np.float32)  # (7, C, J)
    ncand = D.shape[0]
    S = np.empty((ncand, C, J), np.float32)
    for t in range(ncand):
        Vd = V + D[t][:, None, :]
        E = _q8(Vd) - Vd
        S[t] = np.einsum('ck,ckj->cj', P, E * E)
    acc = np.zeros(J, np.float32)
    delta = np.empty((C, J), np.float32)
    for c in range(C):
        tot = S[:, c, :] + MU_BAL * (acc[None, :] + D[:, c, :]) ** 2
        bc = tot.argmin(0)
        delta[c] = np.take_along_axis(D[:, c, :], bc[None, :], 0)[0]
        acc += delta[c]
    return (V + delta[:, None, :]).astype(_FP8)  # (C, K, J) fp8


def _build_program():
    import concourse.bacc as bacc
    import concourse.mybir as mybir
    import concourse.tile as tile
    import concourse.bass as bass

    f32 = mybir.dt.float32
    bf16 = mybir.dt.bfloat16
    fp8 = mybir.dt.float8e4

    nc = bacc.Bacc("TRN2", target_bir_lowering=False, debug=False,
                   num_devices=NCORES)

    # inputs (per-core shapes)
    xg = nc.dram_tensor("xg", [2, DEPTH, 128, NSH], f32, kind="ExternalInput")
    th = nc.dram_tensor("th", [2, 128, 15], f32, kind="ExternalInput")
    # l8[j, cc, p, sp, d, jj] = fp8 of lut_perm[(2*sp+d)*256 + cc*128 + p,
    #                                           j*512 + jj]
    l8 = nc.dram_tensor("l8", [JSLABS, 2, 128, 8, 2, 512], fp8,
                        kind="ExternalInput")
    out = nc.dram_tensor("out", [NCHUNK, 128, JSLABS, 512], bf16,
                         kind="ExternalOutput")

    gt = mybir.AluOpType.is_gt
    eq = mybir.AluOpType.is_equal
    add = mybir.AluOpType.add
    DR = mybir.MatmulPerfMode.DoubleRow

    with tile.TileContext(nc) as tc:
        # keep every pool open for the whole program: early closes let later
        # pools recycle SBUF ranges and inherit WAR waits on whole phases.
        with tc.tile_pool(name="enc", bufs=1) as encp, \
             tc.tile_pool(name="encwork", bufs=1) as wp, \
             tc.tile_pool(name="enctmp", bufs=1) as tp, \
             tc.tile_pool(name="lut", bufs=LUT_BUFS) as lutp, \
             tc.tile_pool(name="part", bufs=1) as pp, \
             tc.tile_pool(name="psum", bufs=6,
                          space=bass.MemorySpace.PSUM) as psp:

            # ---------------- PE warmup -----------------------------------
            wz = wp.tile([128, 2, 128], fp8, tag="wz")
            mz = wp.tile([128, 2, 512], fp8, tag="mz")
            nc.vector.memset(wz[:], 0.0)
            nc.vector.memset(mz[:], 0.0)
            pz = psp.tile([128, 512], f32, tag="ps", name="warm")
            for i in range(N_WARM):
                nc.tensor.matmul(pz[:], wz[:], mz[:],
                                 start=(i == 0), stop=(i == N_WARM - 1),
                                 perf_mode=DR)

            # ---------------- input DMAs (issue order matters) -------------
            tht = []
            xt = []
            lt = {}

            def load_lut(j, cc):
                t = lutp.tile([128, 8, 2, 512], fp8, tag="lut",
                              name=f"l{j}_{cc}")
                nc.sync.dma_start(t[:, 0:4, :, :], l8[j, cc][:, 0:4])
                nc.sync.dma_start(t[:, 4:8, :, :], l8[j, cc][:, 4:8])
                lt[(j, cc)] = t

            def load_x(cc):
                row = []
                t = wp.tile([128, 15], f32, tag=f"th{cc}")
                tht.append(t)
                for l in range(DEPTH):
                    x = wp.tile([128, NSH], f32, tag=f"x{l}_{cc}",
                                name=f"x{l}_{cc}")
                    nc.sync.dma_start(x[:], xg[cc, l])
                    if l == 0:
                        # tiny th transfer slots in behind l0 so the first
                        # compare (needs l0+th) isn't delayed by it
                        nc.sync.dma_start(t[:], th[cc])
                    row.append(x)
                xt.append(row)

            # j0-3's slabs stream right after cc0's x so the PE's m01 sweep
            # is never lut-gated; cc1's x slots in before j4-7 (its encode
            # isn't consumed until the cc1 phase).
            load_x(0)
            for j in range(4):
                load_lut(j, 0)
            load_x(1)
            for j in range(4, JSLABS):
                load_lut(j, 0)
            for j in range(JSLABS):
                load_lut(j, 1)

            # one-hot tiles enc[(cc, off, sp)] for 256-wide n-slices.
            pieces = [(0, 0, 256), (0, 256, 256), (1, 0, 256), (1, 256, 256)]
            enc8 = {}
            for cc, off, w in pieces:
                for sp in range(8):
                    enc8[(cc, off, sp)] = encp.tile(
                        [128, 2, w], fp8, tag=f"e{cc}_{off}_{sp}",
                        name=f"e{cc}_{off}_{sp}")

            def encode_piece(cc, off, w):
                nsl = slice(off, off + w)
                B = [tp.tile([128, 256], bf16, tag=f"b{i}",
                             name=f"b{i}_{cc}{off}")[:, :w]
                     for i in range(15)]
                for i in range(15):
                    nc.vector.tensor_single_scalar(
                        B[i], xt[cc][_level_of_node(i)][:, nsl],
                        tht[cc][:, i:i + 1], gt)

                def mux(u, v, sel):
                    # sel ? v : u, in place: u's tile keeps its value where
                    # !sel (u is dead afterwards at every call site).  The
                    # mask must be integer-typed; bf16 0/1 bitcast to u16
                    # keeps the nonzero-means-copy semantics.
                    nc.vector.copy_predicated(u, sel.bitcast(mybir.dt.uint16), v)
                    return u

                b0 = B[0]
                b1 = mux(B[1], B[2], b0)
                m0 = mux(B[3], B[4], b1)
                m1 = mux(B[5], B[6], b1)
                b2 = mux(m0, m1, b0)
                c00 = mux(B[7], B[8], b2)
                c01 = mux(B[9], B[10], b2)
                c10 = mux(B[11], B[12], b2)
                c11 = mux(B[13], B[14], b2)
                d0 = mux(c00, c01, b1)
                d1 = mux(c10, c11, b1)
                b3 = mux(d0, d1, b0)

                # idx = 8*b0 + 4*b1 + 2*b2 + b3 via in-place Horner on b0
                idx = b0
                nc.vector.tensor_scalar_mul(idx, idx, 2.0)
                nc.vector.tensor_add(idx, idx, b1)
                nc.vector.tensor_scalar_mul(idx, idx, 2.0)
                nc.vector.tensor_add(idx, idx, b2)
                nc.vector.tensor_scalar_mul(idx, idx, 2.0)
                nc.vector.tensor_add(idx, idx, b3)

                for k in range(K):
                    # one-hot planes: back half on GPSIMD so the DVE chain's
                    # tail (which gates the next piece) is shorter
                    eng = nc.vector if k < 10 else nc.gpsimd
                    eng.tensor_single_scalar(
                        enc8[(cc, off, k // 2)][:, k % 2, :], idx,
                        float(k), eq)

            for cc, off, w in pieces:
                encode_piece(cc, off, w)

            # weight slice for (cc, m): the enc piece covering m's n-range
            def wslice(cc, m, sp):
                for pcc, off, w in pieces:
                    if pcc == cc and off <= m * 128 < off + w:
                        o = m * 128 - off
                        return enc8[(cc, off, sp)][:, :, o:o + 128]
                raise KeyError

            # ---------------- matmul passes --------------------------------
            # every (cc, j, m) accumulation closes immediately (1-2 live PSUM
            # banks); cc0 closes to a bf16 partial, cc1 merges partial + PSUM
            # -> bf16 out on DVE.  cc0 order: m01 sweep over j0-3 first (only
            # needs the s0 encode piece), then the m23 backlog, then j4-7.
            # sp5-7's one-hots come from the (lagging) GPSIMD queue, so they
            # go last.
            part = {}
            ORDER = list(range(5)) + list(range(5, 8))

            def jm_tile(cc, j, m):
                ps = psp.tile([128, 512], f32, tag="ps",
                              name=f"ps{cc}_{j}_{m}")
                for i, sp in enumerate(ORDER):
                    nc.tensor.matmul(
                        ps[:], wslice(cc, m, sp),
                        lt[(j, cc)][:, sp, :, :],
                        start=(i == 0), stop=(i == len(ORDER) - 1),
                        perf_mode=DR)
                if cc == 0:
                    pt = pp.tile([128, 512], bf16, tag=f"pt{j}_{m}",
                                 name=f"pt{j}_{m}")
                    part[(j, m)] = pt
                    nc.scalar.copy(pt[:], ps[:])
                else:
                    pt = part[(j, m)]
                    nc.vector.tensor_tensor(pt[:], pt[:], ps[:], add)
                    nc.sync.dma_start(out[m, :, j], pt[:])

            def jm_tile_split(cc, j, m):
                # final tile: two column-half accumulations so the first
                # half's merge + out-DMA overlap the second half's matmuls,
                # shortening the kernel tail
                pt = part[(j, m)]
                for half in range(2):
                    cols = slice(half * 256, (half + 1) * 256)
                    ps = psp.tile([128, 256], f32, tag="psh",
                                  name=f"psh{half}", bufs=2)
                    for i, sp in enumerate(ORDER):
                        nc.tensor.matmul(
                            ps[:], wslice(cc, m, sp),
                            lt[(j, cc)][:, sp, :, cols],
                            start=(i == 0), stop=(i == len(ORDER) - 1),
                            perf_mode=DR)
                    nc.vector.tensor_tensor(pt[:, cols], pt[:, cols],
                                            ps[:], add)
                    nc.sync.dma_start(out[m, :, j][:, cols], pt[:, cols])

            for j in range(4):
                for m in (0, 1):
                    jm_tile(0, j, m)
            for j in range(4):
                for m in (2, 3):
                    jm_tile(0, j, m)
            for j in range(4, JSLABS):
                for m in range(NCHUNK):
                    jm_tile(0, j, m)
            for j in range(JSLABS):
                for m in range(NCHUNK):
                    if j == JSLABS - 1 and m == NCHUNK - 1:
                        jm_tile_split(1, j, m)
                    else:
                        jm_tile(1, j, m)

    nc.compile()
    return nc


_BASE_TREE = np.array([
    [-1,-1,0,-1,0,0,0,-1,0,0,0,0,0,0,0],[-1,-1,0,-1,0,0,0,1,0,0,0,0,0,0,0],
    [-1,-1,0,1,0,0,0,0,-1,0,0,0,0,0,0],[-1,-1,0,1,0,0,0,0,1,0,0,0,0,0,0],
    [-1,1,0,0,-1,0,0,0,0,-1,0,0,0,0,0],[-1,1,0,0,-1,0,0,0,0,1,0,0,0,0,0],
    [-1,1,0,0,1,0,0,0,0,0,-1,0,0,0,0],[-1,1,0,0,1,0,0,0,0,0,1,0,0,0,0],
    [1,0,-1,0,0,-1,0,0,0,0,0,-1,0,0,0],[1,0,-1,0,0,-1,0,0,0,0,0,1,0,0,0],
    [1,0,-1,0,0,1,0,0,0,0,0,0,-1,0,0],[1,0,-1,0,0,1,0,0,0,0,0,0,1,0,0],
    [1,0,1,0,0,0,-1,0,0,0,0,0,0,-1,0],[1,0,1,0,0,0,-1,0,0,0,0,0,0,1,0],
    [1,0,1,0,0,0,1,0,0,0,0,0,0,0,-1],[1,0,1,0,0,0,1,0,0,0,0,0,0,0,1]],
    dtype=np.float32)


def _reference_structure_ok(selection_matrix, tree_des_mat):
    sm = np.asarray(selection_matrix)
    td = np.asarray(tree_des_mat)
    if sm.shape != (C * (K - 1), C * DEPTH) or td.shape != (C * K, C * (K - 1)):
        return False
    base_sel = np.zeros((K - 1, DEPTH), dtype=np.float32)
    base_sel[0, 0] = 1.0
    for i in range(1, K - 1):
        base_sel[i, int(np.log2(i + 1))] = 1.0
    exp_sm = np.zeros_like(sm)
    exp_td = np.ones_like(td)
    for i in range(C):
        exp_sm[i * (K - 1):(i + 1) * (K - 1), i * DEPTH:(i + 1) * DEPTH] = base_sel
        exp_td[i * K:(i + 1) * K, i * (K - 1):(i + 1) * (K - 1)] = _BASE_TREE
    return np.array_equal(sm, exp_sm) and np.array_equal(td, exp_td)


def _numpy_fallback(inputMatrix, dims, selection_matrix, thresholds,
                    tree_des_mat, lut):
    """Faithful numpy replication of the reference forward pass (slow)."""
    x = np.asarray(inputMatrix, np.float32)
    n = x.shape[0]
    c = lut.shape[1]
    chosen = x[:, np.asarray(dims).astype(np.int64)]
    subtracted = (np.asarray(selection_matrix, np.float32) @ chosen.T
                  - np.asarray(thresholds, np.float32))
    sign = np.sign(subtracted).astype(np.float32)
    tree_result = (np.asarray(tree_des_mat, np.float32) @ sign).T.reshape(n, c, K)
    index = np.argmax(tree_result, axis=2)
    onehot = np.eye(K, dtype=np.float32)[index]  # (n, c, K)
    lutm = np.asarray(lut, np.float32).transpose(1, 2, 0).reshape(c * K, -1)
    return (onehot.reshape(n, c * K) @ lutm).astype(np.float32)


def kernel(inputMatrix, dims, selection_matrix, thresholds, tree_des_mat, lut):
    inputMatrix = np.ascontiguousarray(np.asarray(inputMatrix, dtype=np.float32))
    dims_i = np.asarray(dims).astype(np.int64)
    thresholds = np.asarray(thresholds, dtype=np.float32)
    lut = np.asarray(lut, dtype=np.float32)

    if not _reference_structure_ok(selection_matrix, tree_des_mat):
        return _numpy_fallback(inputMatrix, dims_i, selection_matrix,
                               thresholds, tree_des_mat, lut)

    # ---- host prep ----
    chosen = inputMatrix[:, dims_i]  # (N, C*DEPTH)
    th2 = thresholds.reshape(C, K - 1)
    th3 = np.ascontiguousarray(th2.reshape(2, 128, 15))

    # probability-weighted, bias-balanced fp8 quantization: (C, K, J) fp8
    q = _quantize_lut(lut, th2)
    # device layout [j, cc, p, sp, d, jj], ck = (2sp+d)*256 + cc*128 + p
    # q is (C, K, J) = (cc*128+p, 2sp+d, j*512+jj)
    l8_np = np.ascontiguousarray(
        q.reshape(2, 128, 8, 2, JSLABS, 512).transpose(4, 0, 1, 2, 3, 5))

    from concourse.bass_utils import run_bass_kernel_spmd

    if "nc" not in _CACHED:
        _CACHED["nc"] = _build_program()
    nc = _CACHED["nc"]

    in_maps = []
    for g in range(NCORES):
        ch = chosen[g * NSH:(g + 1) * NSH].reshape(NSH, 2, 128, DEPTH)
        xg_np = np.ascontiguousarray(ch.transpose(1, 3, 2, 0))  # [cc, l, p, n]
        in_maps.append({"xg": xg_np, "th": th3, "l8": l8_np})

    res = run_bass_kernel_spmd(nc, in_maps, list(range(NCORES)))
    out = np.concatenate(
        [np.asarray(res.results[g]["out"]).astype(np.float32)
         .reshape(NSH, OUT_FEATURES) for g in range(NCORES)], axis=0)
    return out


# revision 30
# speedup vs baseline: 1.4685x; 1.0485x over previous
"""NimbusLinear (VQ codebook) Trainium2 kernel, v2.

Math: the reference's selection/threshold/sign/tree_des_mat/softmax/argmax
chain is exactly a depth-4 binary-tree threshold descent per (row, codeblock):
  node j at level l compares chosen[n, c*4+l] > thresholds[c*15+j]
  leaf index -> one-hot Encoded[n, c*16+k]
and the final einsum is a dense matmul out = Encoded @ lut_perm with
lut_perm[k*256+c, j] = lut[j, c, k].

Device strategy (8 cores, data-parallel over N rows, 512 rows/core, no
collectives):
  - encode: 15 exact-fp32 threshold compares + mux-tree descent + one-hot
    eq's on DVE (tail eq's on GPSIMD), in 256-row n-slices.
  - matmul: SINGLE fp8 pass (16 DoubleRow matmuls per 128x512 output tile,
    contracting 256 rows each at 0.5 cycles/output-col).  No hi/lo split:
    the lut is quantized with probability-weighted, bias-balanced rounding
    (below), which matches the old hi+partial-lo error at 16/26 of the PE
    work and 16MB instead of 26MB of lut DMA.
  - quantization: for each (codeblock c, out-column j) the 16 bucket values
    share one continuous pre-quantization shift delta[c,j]; out[n,j] then
    carries sum_c delta[c,j] as a per-j constant plus per-entry grid error.
    delta is chosen per (c,j) to put the highest-probability bucket (bucket
    probabilities are computed analytically from the thresholds, since the
    tree compares N(0,1) features) exactly on the fp8 e4m3 grid and to
    minimize the probability-weighted grid error of the rest; an
    error-diffusion scan over c keeps |sum_c delta| small so no correction
    term is needed on device.
  - two cc passes: the cc0 pass closes each (j, m) PSUM tile immediately to a
    bf16 partial in SBUF (1-2 live banks instead of 8+, so PE order is free);
    the cc1 pass merges partial + PSUM -> bf16 out on DVE.
  - PE warmup: dummy DoubleRow matmuls keep the PE busy early so the p-state
    ramp (0.65/1.2GHz for the first 3us) is spent before real work arrives.
  - DMA issue order keeps the PE fed: th, first-half xg, cc0 lut slabs
    (kp-halved), second-half xg, cc1 lut slabs.
"""

import sys

sys.path.insert(0, "/opt/trn_rl_repo")

import numpy as np
import ml_dtypes

K = 16
DEPTH = 4
C = 256
IN_FEATURES = 4096
OUT_FEATURES = 4096
N_ROWS = 4096
NCORES = 8
NSH = N_ROWS // NCORES  # 512 rows per core
NCHUNK = NSH // 128  # 4 partition chunks of rows per core
JSLABS = OUT_FEATURES // 512  # 8 output column slabs
LUT_BUFS = 12  # in-flight lut slab tiles (8KB/partition each)
N_WARM = 66  # PE warmup dummy matmuls
MU_BAL = 1e-2  # bias-balance weight in the quantizer
# per-piece count of one-hot eq ops on DVE (rest go to GPSIMD)
N_DVE_EQ = [12, 6, 6, 2, 2]
N_SPLIT = 1  # how many final cc1 tiles use the column-split tail
N_ALT = 8  # how many final cc1 outs alternate ACT/SP sequencers

_CACHED = {}

_FP8 = ml_dtypes.float8_e4m3


def _level_of_node(i):
    return int(np.floor(np.log2(i + 1)))


def _phi(x):
    """Standard normal CDF, vectorized (Abramowitz-Stegun 7.1.26 erf)."""
    z = np.abs(x) / np.sqrt(2.0)
    t = 1.0 / (1.0 + 0.3275911 * z)
    poly = t * (0.254829592 + t * (-0.284496736 + t * (1.421413741
               + t * (-1.453152027 + t * 1.061405429))))
    erf = 1.0 - poly * np.exp(-z * z)
    return np.where(x >= 0, 0.5 * (1.0 + erf), 0.5 * (1.0 - erf))


def _bucket_probs(th):
    """th: (C, 15) thresholds.  Returns (C, K) analytic leaf probabilities
    for N(0,1) features descending the depth-4 tree."""
    Phi = _phi(th.astype(np.float64))
    P = np.zeros((C, K))
    for k in range(K):
        bits = [(k >> 3) & 1, (k >> 2) & 1, (k >> 1) & 1, k & 1]
        node = 0
        p = np.ones(C)
        for b in bits:
            pt = Phi[:, node]
            p = p * ((1.0 - pt) if b else pt)
            node = 2 * node + (2 if b else 1)
        P[:, k] = p
    return P.astype(np.float32)


def _q8(a):
    return a.astype(_FP8).astype(np.float32)


def _quantize_lut(lut, th):
    """Probability-weighted, bias-balanced fp8 quantization.

    lut: (J, C, K) f32.  Returns (C, K, J) fp8 values.
    For each (c, j): pick shift delta from grid points near the
    highest-probability bucket's value, minimizing
      sum_k P[c,k] * (Q(v_k+delta) - (v_k+delta))^2  +  MU_BAL*(running bias)^2
    scanning c in order so sum_c delta[c,j] stays near zero (the delta sum
    appears in every output row as a per-j constant, so it must stay small).
    """
    P = _bucket_probs(th)
    V = np.ascontiguousarray(lut.transpose(1, 2, 0))  # (C, K, J)
    J = V.shape[2]
    kstar = np.argmax(P, axis=1)
    Vs = V[np.arange(C), kstar, :]  # (C, J)
    ulp = np.maximum(np.abs(Vs) * 2.0 ** -3, 2.0 ** -9)
    D = []
    for t in (-2.0, -1.0, -0.45, 0.0, 0.45, 1.0, 2.0):
        g = _q8(Vs + t * ulp)
        d = g - Vs
        # guard against grid jumps across exponent/sign boundaries
        d = np.where(np.abs(d) > 0.6 * np.abs(Vs) + 0.1, 0.0, d)
        D.append(d)
    D = np.stack(D, 0).astype(np.float32)  # (7, C, J)
    ncand = D.shape[0]
    S = np.empty((ncand, C, J), np.float32)
    for t in range(ncand):
        Vd = V + D[t][:, None, :]
        E = _q8(Vd) - Vd
        S[t] = np.einsum('ck,ckj->cj', P, E * E)
    acc = np.zeros(J, np.float32)
    delta = np.empty((C, J), np.float32)
    for c in range(C):
        tot = S[:, c, :] + MU_BAL * (acc[None, :] + D[:, c, :]) ** 2
        bc = tot.argmin(0)
        delta[c] = np.take_along_axis(D[:, c, :], bc[None, :], 0)[0]
        acc += delta[c]
    return (V + delta[:, None, :]).astype(_FP8)  # (C, K, J) fp8


def _build_program():
    import concourse.bacc as bacc
    import concourse.mybir as mybir
    import concourse.tile as tile
    import concourse.bass as bass

    f32 = mybir.dt.float32
    bf16 = mybir.dt.bfloat16
    fp8 = mybir.dt.float8e4

    nc = bacc.Bacc("TRN2", target_bir_lowering=False, debug=False,
                   num_devices=NCORES)

    # inputs (per-core shapes)
    xg = nc.dram_tensor("xg", [2, DEPTH, 128, NSH], f32, kind="ExternalInput")
    th = nc.dram_tensor("th", [2, 128, 15], f32, kind="ExternalInput")
    # l8[j, cc, p, sp, d, jj] = fp8 of lut_perm[(2*sp+d)*256 + cc*128 + p,
    #                                           j*512 + jj]
    l8 = nc.dram_tensor("l8", [JSLABS, 2, 128, 8, 2, 512], fp8,
                        kind="ExternalInput")
    out = nc.dram_tensor("out", [NCHUNK, 128, JSLABS, 512], bf16,
                         kind="ExternalOutput")

    gt = mybir.AluOpType.is_gt
    eq = mybir.AluOpType.is_equal
    add = mybir.AluOpType.add
    DR = mybir.MatmulPerfMode.DoubleRow

    with tile.TileContext(nc) as tc:
        # keep every pool open for the whole program: early closes let later
        # pools recycle SBUF ranges and inherit WAR waits on whole phases.
        with tc.tile_pool(name="enc", bufs=1) as encp, \
             tc.tile_pool(name="encwork", bufs=1) as wp, \
             tc.tile_pool(name="enctmp", bufs=1) as tp, \
             tc.tile_pool(name="lut", bufs=LUT_BUFS) as lutp, \
             tc.tile_pool(name="part", bufs=1) as pp, \
             tc.tile_pool(name="psum", bufs=6,
                          space=bass.MemorySpace.PSUM) as psp:

            # ---------------- PE warmup -----------------------------------
            wz = wp.tile([128, 2, 128], fp8, tag="wz")
            mz = wp.tile([128, 2, 512], fp8, tag="mz")
            nc.vector.memset(wz[:], 0.0)
            nc.vector.memset(mz[:], 0.0)
            pz = psp.tile([128, 512], f32, tag="ps", name="warm")
            for i in range(N_WARM):
                nc.tensor.matmul(pz[:], wz[:], mz[:],
                                 start=(i == 0), stop=(i == N_WARM - 1),
                                 perf_mode=DR)

            # ---------------- input DMAs (issue order matters) -------------
            tht = []
            xt = []
            lt = {}

            def load_lut(j, cc):
                t = lutp.tile([128, 8, 2, 512], fp8, tag="lut",
                              name=f"l{j}_{cc}")
                nc.sync.dma_start(t[:, 0:4, :, :], l8[j, cc][:, 0:4])
                nc.sync.dma_start(t[:, 4:8, :, :], l8[j, cc][:, 4:8])
                lt[(j, cc)] = t

            def load_x(cc):
                row = []
                t = wp.tile([128, 15], f32, tag=f"th{cc}")
                tht.append(t)
                for l in range(DEPTH):
                    x = wp.tile([128, NSH], f32, tag=f"x{l}_{cc}",
                                name=f"x{l}_{cc}")
                    nc.sync.dma_start(x[:], xg[cc, l])
                    if l == 0:
                        # tiny th transfer slots in behind l0 so the first
                        # compare (needs l0+th) isn't delayed by it
                        nc.sync.dma_start(t[:], th[cc])
                    row.append(x)
                xt.append(row)

            # j0-3's slabs stream right after cc0's x so the PE's m01 sweep
            # is never lut-gated; cc1's x slots in before j4-7 (its encode
            # isn't consumed until the cc1 phase).
            load_x(0)
            for j in range(4):
                load_lut(j, 0)
            load_x(1)
            for j in range(4, JSLABS):
                load_lut(j, 0)
            for j in range(JSLABS):
                load_lut(j, 1)

            # one-hot tiles enc[(cc, off, sp)] per encode n-slice.  cc0's
            # second half is split into two 128-wide pieces so the m2/m3
            # matmul work opens as early as possible (it fills the PE gap
            # between the end of the m01 sweep and the j4-7 slab arrivals).
            pieces = [(0, 0, 256), (0, 256, 128), (0, 384, 128),
                      (1, 0, 256), (1, 256, 256)]
            enc8 = {}
            for cc, off, w in pieces:
                for sp in range(8):
                    enc8[(cc, off, sp)] = encp.tile(
                        [128, 2, w], fp8, tag=f"e{cc}_{off}_{sp}",
                        name=f"e{cc}_{off}_{sp}")

            def encode_piece(cc, off, w, n_dve_eq, pi):
                """Tree descent via threshold muxing: instead of 15 compares
                (one per node), materialize the per-(c,n) selected threshold
                for each level with predicated copies of the threshold
                columns (cheap, and the materializations only depend on th,
                not x), then compare once per level.  One-hot planes go to
                GPSIMD except the first n_dve_eq (which gate the matmuls)."""
                nsl = slice(off, off + w)
                u32 = mybir.dt.uint32

                def bcol(i):
                    # threshold column i broadcast along the free dim
                    return tht[cc][:, i:i + 1].broadcast_to([128, w])

                # cc0 pieces gate the PE start: their threshold mats run on
                # the (otherwise idle until the closes) ACT engine, with
                # per-piece buffers so ACT isn't WAR-stalled on the previous
                # piece's DVE reads.  cc1 pieces mat on DVE (no rush).
                # Everything stays f32: rounding the selected thresholds to
                # bf16 flips ~0.2% of the compares into wrong buckets.
                mat_eng = nc.scalar if pi < 3 else nc.vector
                tsuf = f"p{pi}" if pi < 3 else ""

                def btile(name):
                    return tp.tile([128, w], f32, tag=f"{name}w{w}",
                                   name=f"{name}_{cc}{off}")

                def mtile(name):
                    t = tp.tile([128, w], f32, tag=f"{name}{tsuf}w{w}",
                                name=f"{name}{tsuf}_{cc}{off}")
                    if mat_eng is nc.scalar:
                        nc.scalar.copy(t, bcol(int(name[1:])))
                    else:
                        nc.vector.tensor_copy(t, bcol(int(name[1:])))
                    return t

                # threshold materializations (x-independent, run during the
                # x DMA): level-1 selectee + 2 level-2 + 4 level-3 pairs
                t1 = mtile("t1")
                t3 = mtile("t3")
                t5 = mtile("t5")
                t7 = mtile("t7")
                t9 = mtile("t9")
                t11 = mtile("t11")
                t13 = mtile("t13")

                def pred(u, sel, v):
                    # sel ? v : u, in place on u
                    nc.vector.copy_predicated(u, sel.bitcast(u32), v)
                    return u

                # per-piece tag: this tile becomes idx, which the GPSIMD eq
                # queue keeps reading long after the DVE moved on — a shared
                # tag would stall the next piece's first compare on it
                b0 = tp.tile([128, w], f32, tag=f"b0p{pi}",
                             name=f"b0p{pi}")
                nc.vector.tensor_single_scalar(
                    b0, xt[cc][0][:, nsl], tht[cc][:, 0:1], gt)
                # level 1: t_sel = b0 ? th[2] : th[1]
                pred(t1, b0, bcol(2))
                b1 = btile("b1")
                nc.vector.tensor_tensor(b1, xt[cc][1][:, nsl], t1, gt)
                # level 2: th[3..6] by (b0, b1)
                pred(t3, b1, bcol(4))
                pred(t5, b1, bcol(6))
                pred(t3, b0, t5)
                b2 = btile("b2")
                nc.vector.tensor_tensor(b2, xt[cc][2][:, nsl], t3, gt)
                # level 3: th[7..14] by (b0, b1, b2)
                pred(t7, b2, bcol(8))
                pred(t9, b2, bcol(10))
                pred(t11, b2, bcol(12))
                pred(t13, b2, bcol(14))
                pred(t7, b1, t9)
                pred(t11, b1, t13)
                pred(t7, b0, t11)
                b3 = btile("b3")
                nc.vector.tensor_tensor(b3, xt[cc][3][:, nsl], t7, gt)

                # idx = 8*b0 + 4*b1 + 2*b2 + b3, Horner with fused
                # (in0*2 + in1) scalar_tensor_tensor steps, in place on b0
                idx = b0
                mult = mybir.AluOpType.mult
                nc.vector.scalar_tensor_tensor(idx, idx, 2.0, b1, mult, add)
                nc.vector.scalar_tensor_tensor(idx, idx, 2.0, b2, mult, add)
                nc.vector.scalar_tensor_tensor(idx, idx, 2.0, b3, mult, add)

                for k in range(K):
                    eng = nc.vector if k < n_dve_eq else nc.gpsimd
                    eng.tensor_single_scalar(
                        enc8[(cc, off, k // 2)][:, k % 2, :], idx,
                        float(k), eq)

            # early pieces' one-hot planes gate matmul sweeps, so most of
            # their eq's run on the (fast) DVE and GPSIMD only takes the
            # last pairs; cc1 pieces have slack, GPSIMD takes almost all
            for pi, (cc, off, w) in enumerate(pieces):
                encode_piece(cc, off, w, n_dve_eq=N_DVE_EQ[pi], pi=pi)

            # weight slice for (cc, m): the enc piece covering m's n-range
            def wslice(cc, m, sp):
                for pcc, off, w in pieces:
                    if pcc == cc and off <= m * 128 < off + w:
                        o = m * 128 - off
                        return enc8[(cc, off, sp)][:, :, o:o + 128]
                raise KeyError

            # ---------------- matmul passes --------------------------------
            # every (cc, j, m) accumulation closes immediately (1-2 live PSUM
            # banks); cc0 closes to a bf16 partial, cc1 merges partial + PSUM
            # -> bf16 out on DVE.  cc0 order: m01 sweep over j0-3 first (only
            # needs the s0 encode piece), then the m23 backlog, then j4-7.
            # sp5-7's one-hots come from the (lagging) GPSIMD queue, so they
            # go last.
            part = {}
            ORDER = list(range(5)) + list(range(5, 8))

            def jm_tile(cc, j, m, out_eng=None):
                ps = psp.tile([128, 512], f32, tag="ps",
                              name=f"ps{cc}_{j}_{m}")
                for i, sp in enumerate(ORDER):
                    nc.tensor.matmul(
                        ps[:], wslice(cc, m, sp),
                        lt[(j, cc)][:, sp, :, :],
                        start=(i == 0), stop=(i == len(ORDER) - 1),
                        perf_mode=DR)
                if cc == 0:
                    pt = pp.tile([128, 512], bf16, tag=f"pt{j}_{m}",
                                 name=f"pt{j}_{m}")
                    part[(j, m)] = pt
                    nc.scalar.copy(pt[:], ps[:])
                else:
                    pt = part[(j, m)]
                    nc.vector.tensor_tensor(pt[:], pt[:], ps[:], add)
                    # out-DMAs go through the ACT sequencer: the SP queue is
                    # head-of-line blocked on lut dma_starts waiting for slab
                    # buffers, which would bunch every out into the tail
                    (out_eng or nc.scalar).dma_start(out[m, :, j], pt[:])

            def jm_tile_split(cc, j, m, si):
                # tail tiles: two column-half accumulations so the first
                # half's merge + out-DMA overlap the second half's matmuls,
                # and the outs alternate between the ACT and SP sequencers
                # (a single DGE queue costs ~667ns per out, serializing the
                # final drain)
                pt = part[(j, m)]
                for half in range(2):
                    cols = slice(half * 256, (half + 1) * 256)
                    ps = psp.tile([128, 256], f32, tag="psh",
                                  name=f"psh{j}_{m}_{half}", bufs=2)
                    for i, sp in enumerate(ORDER):
                        nc.tensor.matmul(
                            ps[:], wslice(cc, m, sp),
                            lt[(j, cc)][:, sp, :, cols],
                            start=(i == 0), stop=(i == len(ORDER) - 1),
                            perf_mode=DR)
                    nc.vector.tensor_tensor(pt[:, cols], pt[:, cols],
                                            ps[:], add)
                    eng = nc.scalar if (si + half) % 2 == 0 else nc.sync
                    eng.dma_start(out[m, :, j][:, cols], pt[:, cols])

            for j in range(4):
                for m in (0, 1):
                    jm_tile(0, j, m)
            for m in (2, 3):  # m2 opens with piece C1, m3 with C2
                for j in range(4):
                    jm_tile(0, j, m)
            for j in range(4, JSLABS):
                for m in range(NCHUNK):
                    jm_tile(0, j, m)
            n_cc1 = JSLABS * NCHUNK
            si = 0
            for t, (j, m) in enumerate((j, m) for j in range(JSLABS)
                                       for m in range(NCHUNK)):
                alt = t >= n_cc1 - N_ALT
                if t >= n_cc1 - N_SPLIT:
                    jm_tile_split(1, j, m, si=si)
                    si += 2
                else:
                    eng = None
                    if alt:
                        eng = nc.scalar if si % 2 == 0 else nc.sync
                        si += 1
                    jm_tile(1, j, m, out_eng=eng)

    nc.compile()
    return nc


_BASE_TREE = np.array([
    [-1,-1,0,-1,0,0,0,-1,0,0,0,0,0,0,0],[-1,-1,0,-1,0,0,0,1,0,0,0,0,0,0,0],
    [-1,-1,0,1,0,0,0,0,-1,0,0,0,0,0,0],[-1,-1,0,1,0,0,0,0,1,0,0,0,0,0,0],
    [-1,1,0,0,-1,0,0,0,0,-1,0,0,0,0,0],[-1,1,0,0,-1,0,0,0,0,1,0,0,0,0,0],
    [-1,1,0,0,1,0,0,0,0,0,-1,0,0,0,0],[-1,1,0,0,1,0,0,0,0,0,1,0,0,0,0],
    [1,0,-1,0,0,-1,0,0,0,0,0,-1,0,0,0],[1,0,-1,0,0,-1,0,0,0,0,0,1,0,0,0],
    [1,0,-1,0,0,1,0,0,0,0,0,0,-1,0,0],[1,0,-1,0,0,1,0,0,0,0,0,0,1,0,0],
    [1,0,1,0,0,0,-1,0,0,0,0,0,0,-1,0],[1,0,1,0,0,0,-1,0,0,0,0,0,0,1,0],
    [1,0,1,0,0,0,1,0,0,0,0,0,0,0,-1],[1,0,1,0,0,0,1,0,0,0,0,0,0,0,1]],
    dtype=np.float32)


def _reference_structure_ok(selection_matrix, tree_des_mat):
    sm = np.asarray(selection_matrix)
    td = np.asarray(tree_des_mat)
    if sm.shape != (C * (K - 1), C * DEPTH) or td.shape != (C * K, C * (K - 1)):
        return False
    base_sel = np.zeros((K - 1, DEPTH), dtype=np.float32)
    base_sel[0, 0] = 1.0
    for i in range(1, K - 1):
        base_sel[i, int(np.log2(i + 1))] = 1.0
    exp_sm = np.zeros_like(sm)
    exp_td = np.ones_like(td)
    for i in range(C):
        exp_sm[i * (K - 1):(i + 1) * (K - 1), i * DEPTH:(i + 1) * DEPTH] = base_sel
        exp_td[i * K:(i + 1) * K, i * (K - 1):(i + 1) * (K - 1)] = _BASE_TREE
    return np.array_equal(sm, exp_sm) and np.array_equal(td, exp_td)


def _numpy_fallback(inputMatrix, dims, selection_matrix, thresholds,
                    tree_des_mat, lut):
    """Faithful numpy replication of the reference forward pass (slow)."""
    x = np.asarray(inputMatrix, np.float32)
    n = x.shape[0]
    c = lut.shape[1]
    chosen = x[:, np.asarray(dims).astype(np.int64)]
    subtracted = (np.asarray(selection_matrix, np.float32) @ chosen.T
                  - np.asarray(thresholds, np.float32))
    sign = np.sign(subtracted).astype(np.float32)
    tree_result = (np.asarray(tree_des_mat, np.float32) @ sign).T.reshape(n, c, K)
    index = np.argmax(tree_result, axis=2)
    onehot = np.eye(K, dtype=np.float32)[index]  # (n, c, K)
    lutm = np.asarray(lut, np.float32).transpose(1, 2, 0).reshape(c * K, -1)
    return (onehot.reshape(n, c * K) @ lutm).astype(np.float32)


def kernel(inputMatrix, dims, selection_matrix, thresholds, tree_des_mat, lut):
    inputMatrix = np.ascontiguousarray(np.asarray(inputMatrix, dtype=np.float32))
    dims_i = np.asarray(dims).astype(np.int64)
    thresholds = np.asarray(thresholds, dtype=np.float32)
    lut = np.asarray(lut, dtype=np.float32)

    if not _reference_structure_ok(selection_matrix, tree_des_mat):
        return _numpy_fallback(inputMatrix, dims_i, selection_matrix,
                               thresholds, tree_des_mat, lut)

    # ---- host prep ----
    chosen = inputMatrix[:, dims_i]  # (N, C*DEPTH)
    th2 = thresholds.reshape(C, K - 1)
    th3 = np.ascontiguousarray(th2.reshape(2, 128, 15))

    # probability-weighted, bias-balanced fp8 quantization: (C, K, J) fp8
    q = _quantize_lut(lut, th2)
    # device layout [j, cc, p, sp, d, jj], ck = (2sp+d)*256 + cc*128 + p
    # q is (C, K, J) = (cc*128+p, 2sp+d, j*512+jj)
    l8_np = np.ascontiguousarray(
        q.reshape(2, 128, 8, 2, JSLABS, 512).transpose(4, 0, 1, 2, 3, 5))

    from concourse.bass_utils import run_bass_kernel_spmd

    if "nc" not in _CACHED:
        _CACHED["nc"] = _build_program()
    nc = _CACHED["nc"]

    in_maps = []
    for g in range(NCORES):
        ch = chosen[g * NSH:(g + 1) * NSH].reshape(NSH, 2, 128, DEPTH)
        xg_np = np.ascontiguousarray(ch.transpose(1, 3, 2, 0))  # [cc, l, p, n]
        in_maps.append({"xg": xg_np, "th": th3, "l8": l8_np})

    res = run_bass_kernel_spmd(nc, in_maps, list(range(NCORES)))
    out = np.concatenate(
        [np.asarray(res.results[g]["out"]).astype(np.float32)
         .reshape(NSH, OUT_FEATURES) for g in range(NCORES)], axis=0)
    return out


# revision 41
# speedup vs baseline: 1.5221x; 1.0365x over previous
"""NimbusLinear (VQ codebook) Trainium2 kernel, v2.

Math: the reference's selection/threshold/sign/tree_des_mat/softmax/argmax
chain is exactly a depth-4 binary-tree threshold descent per (row, codeblock):
  node j at level l compares chosen[n, c*4+l] > thresholds[c*15+j]
  leaf index -> one-hot Encoded[n, c*16+k]
and the final einsum is a dense matmul out = Encoded @ lut_perm with
lut_perm[k*256+c, j] = lut[j, c, k].

Device strategy (8 cores, data-parallel over N rows, 512 rows/core, no
collectives):
  - encode: 15 exact-fp32 threshold compares + mux-tree descent + one-hot
    eq's on DVE (tail eq's on GPSIMD), in 256-row n-slices.
  - matmul: SINGLE fp8 pass (16 DoubleRow matmuls per 128x512 output tile,
    contracting 256 rows each at 0.5 cycles/output-col).  No hi/lo split:
    the lut is quantized with probability-weighted, bias-balanced rounding
    (below), which matches the old hi+partial-lo error at 16/26 of the PE
    work and 16MB instead of 26MB of lut DMA.
  - quantization: for each (codeblock c, out-column j) the 16 bucket values
    share one continuous pre-quantization shift delta[c,j]; out[n,j] then
    carries sum_c delta[c,j] as a per-j constant plus per-entry grid error.
    delta is chosen per (c,j) to put the highest-probability bucket (bucket
    probabilities are computed analytically from the thresholds, since the
    tree compares N(0,1) features) exactly on the fp8 e4m3 grid and to
    minimize the probability-weighted grid error of the rest; an
    error-diffusion scan over c keeps |sum_c delta| small so no correction
    term is needed on device.
  - two cc passes: the cc0 pass closes each (j, m) PSUM tile immediately to a
    bf16 partial in SBUF (1-2 live banks instead of 8+, so PE order is free);
    the cc1 pass merges partial + PSUM -> bf16 out on DVE.
  - PE warmup: dummy DoubleRow matmuls keep the PE busy early so the p-state
    ramp (0.65/1.2GHz for the first 3us) is spent before real work arrives.
  - DMA issue order keeps the PE fed: th, first-half xg, cc0 lut slabs
    (kp-halved), second-half xg, cc1 lut slabs.
"""

import sys

sys.path.insert(0, "/opt/trn_rl_repo")

import numpy as np
import ml_dtypes

K = 16
DEPTH = 4
C = 256
IN_FEATURES = 4096
OUT_FEATURES = 4096
N_ROWS = 4096
NCORES = 8
NSH = N_ROWS // NCORES  # 512 rows per core
NCHUNK = NSH // 128  # 4 partition chunks of rows per core
JSLABS = OUT_FEATURES // 512  # 8 output column slabs
LUT_BUFS = 12  # in-flight lut slab tiles (8KB/partition each)
N_WARM = 72  # PE warmup dummy matmuls
MU_BAL = 1e-2  # bias-balance weight in the quantizer
# encode n-slices: (cc, col offset, width).  cc0's tail is split into two
# 128-wide pieces so the m2/m3 matmul sweeps open as early as possible.
PIECES = [(0, 0, 256), (0, 256, 128), (0, 384, 128), (1, 0, 256),
          (1, 256, 256)]
# per-piece count of one-hot eq ops on DVE (rest go to GPSIMD)
N_DVE_EQ = [12, 8, 16, 2, 16]
N_SPLIT = 1  # how many final cc1 tiles use the column-split tail
N_ALT = 12  # how many final cc1 outs rotate across DGE queues

_CACHED = {}

_FP8 = ml_dtypes.float8_e4m3


def _level_of_node(i):
    return int(np.floor(np.log2(i + 1)))


def _phi(x):
    """Standard normal CDF, vectorized (Abramowitz-Stegun 7.1.26 erf)."""
    z = np.abs(x) / np.sqrt(2.0)
    t = 1.0 / (1.0 + 0.3275911 * z)
    poly = t * (0.254829592 + t * (-0.284496736 + t * (1.421413741
               + t * (-1.453152027 + t * 1.061405429))))
    erf = 1.0 - poly * np.exp(-z * z)
    return np.where(x >= 0, 0.5 * (1.0 + erf), 0.5 * (1.0 - erf))


def _bucket_probs(th):
    """th: (C, 15) thresholds.  Returns (C, K) analytic leaf probabilities
    for N(0,1) features descending the depth-4 tree."""
    Phi = _phi(th.astype(np.float64))
    P = np.zeros((C, K))
    for k in range(K):
        bits = [(k >> 3) & 1, (k >> 2) & 1, (k >> 1) & 1, k & 1]
        node = 0
        p = np.ones(C)
        for b in bits:
            pt = Phi[:, node]
            p = p * ((1.0 - pt) if b else pt)
            node = 2 * node + (2 if b else 1)
        P[:, k] = p
    return P.astype(np.float32)


def _q8(a):
    return a.astype(_FP8).astype(np.float32)


def _quantize_lut(lut, th):
    """Probability-weighted, bias-balanced fp8 quantization.

    lut: (J, C, K) f32.  Returns (C, K, J) fp8 values.
    For each (c, j): pick shift delta from grid points near the
    highest-probability bucket's value, minimizing
      sum_k P[c,k] * (Q(v_k+delta) - (v_k+delta))^2  +  MU_BAL*(running bias)^2
    scanning c in order so sum_c delta[c,j] stays near zero (the delta sum
    appears in every output row as a per-j constant, so it must stay small).
    """
    P = _bucket_probs(th)
    V = np.ascontiguousarray(lut.transpose(1, 2, 0))  # (C, K, J)
    J = V.shape[2]
    kstar = np.argmax(P, axis=1)
    Vs = V[np.arange(C), kstar, :]  # (C, J)
    ulp = np.maximum(np.abs(Vs) * 2.0 ** -3, 2.0 ** -9)
    D = []
    for t in (-2.0, -1.0, -0.45, 0.0, 0.45, 1.0, 2.0):
        g = _q8(Vs + t * ulp)
        d = g - Vs
        # guard against grid jumps across exponent/sign boundaries
        d = np.where(np.abs(d) > 0.6 * np.abs(Vs) + 0.1, 0.0, d)
        D.append(d)
    D = np.stack(D, 0).astype(np.float32)  # (7, C, J)
    ncand = D.shape[0]
    S = np.empty((ncand, C, J), np.float32)
    for t in range(ncand):
        Vd = V + D[t][:, None, :]
        E = _q8(Vd) - Vd
        S[t] = np.einsum('ck,ckj->cj', P, E * E)
    acc = np.zeros(J, np.float32)
    delta = np.empty((C, J), np.float32)
    for c in range(C):
        tot = S[:, c, :] + MU_BAL * (acc[None, :] + D[:, c, :]) ** 2
        bc = tot.argmin(0)
        delta[c] = np.take_along_axis(D[:, c, :], bc[None, :], 0)[0]
        acc += delta[c]
    return (V + delta[:, None, :]).astype(_FP8)  # (C, K, J) fp8


def _build_program():
    import concourse.bacc as bacc
    import concourse.mybir as mybir
    import concourse.tile as tile
    import concourse.bass as bass

    f32 = mybir.dt.float32
    bf16 = mybir.dt.bfloat16
    fp8 = mybir.dt.float8e4

    nc = bacc.Bacc("TRN2", target_bir_lowering=False, debug=False,
                   num_devices=NCORES)

    # inputs (per-core shapes)
    xg = nc.dram_tensor("xg", [2, DEPTH, 128, NSH], f32, kind="ExternalInput")
    th = nc.dram_tensor("th", [2, 128, 15], f32, kind="ExternalInput")
    # l8[j, cc, p, sp, d, jj] = fp8 of lut_perm[(2*sp+d)*256 + cc*128 + p,
    #                                           j*512 + jj]
    l8 = nc.dram_tensor("l8", [JSLABS, 2, 128, 8, 2, 512], fp8,
                        kind="ExternalInput")
    out = nc.dram_tensor("out", [NCHUNK, 128, JSLABS, 512], bf16,
                         kind="ExternalOutput")

    gt = mybir.AluOpType.is_gt
    eq = mybir.AluOpType.is_equal
    add = mybir.AluOpType.add
    DR = mybir.MatmulPerfMode.DoubleRow

    with tile.TileContext(nc) as tc:
        # keep every pool open for the whole program: early closes let later
        # pools recycle SBUF ranges and inherit WAR waits on whole phases.
        with tc.tile_pool(name="enc", bufs=1) as encp, \
             tc.tile_pool(name="encwork", bufs=1) as wp, \
             tc.tile_pool(name="enctmp", bufs=1) as tp, \
             tc.tile_pool(name="lut", bufs=LUT_BUFS) as lutp, \
             tc.tile_pool(name="part", bufs=1) as pp, \
             tc.tile_pool(name="psum", bufs=6,
                          space=bass.MemorySpace.PSUM) as psp:

            # ---------------- PE warmup -----------------------------------
            wz = wp.tile([128, 2, 128], fp8, tag="wz")
            mz = wp.tile([128, 2, 512], fp8, tag="mz")
            nc.vector.memset(wz[:], 0.0)
            nc.vector.memset(mz[:], 0.0)
            pz = psp.tile([128, 512], f32, tag="ps", name="warm")
            for i in range(N_WARM):
                nc.tensor.matmul(pz[:], wz[:], mz[:],
                                 start=(i == 0), stop=(i == N_WARM - 1),
                                 perf_mode=DR)

            # ---------------- input DMAs (issue order matters) -------------
            tht = []
            xt = []
            lt = {}

            def load_lut(j, cc):
                t = lutp.tile([128, 8, 2, 512], fp8, tag="lut",
                              name=f"l{j}_{cc}")
                nc.sync.dma_start(t[:, 0:4, :, :], l8[j, cc][:, 0:4])
                nc.sync.dma_start(t[:, 4:8, :, :], l8[j, cc][:, 4:8])
                lt[(j, cc)] = t

            def load_x(cc):
                row = []
                t = wp.tile([128, 15], f32, tag=f"th{cc}")
                tht.append(t)
                for l in range(DEPTH):
                    x = wp.tile([128, NSH], f32, tag=f"x{l}_{cc}",
                                name=f"x{l}_{cc}")
                    nc.sync.dma_start(x[:], xg[cc, l])
                    if l == 0:
                        # tiny th transfer slots in behind l0 so the first
                        # compare (needs l0+th) isn't delayed by it
                        nc.sync.dma_start(t[:], th[cc])
                    row.append(x)
                xt.append(row)

            # j0-3's slabs stream right after cc0's x so the PE's m01 sweep
            # is never lut-gated; cc1's x slots in before j4-7 (its encode
            # isn't consumed until the cc1 phase).
            load_x(0)
            for j in range(4):
                load_lut(j, 0)
            load_x(1)
            for j in range(4, JSLABS):
                load_lut(j, 0)
            for j in range(JSLABS):
                load_lut(j, 1)

            # one-hot tiles enc[(cc, off, sp)] per encode n-slice
            pieces = PIECES
            enc8 = {}
            for cc, off, w in pieces:
                for sp in range(8):
                    enc8[(cc, off, sp)] = encp.tile(
                        [128, 2, w], fp8, tag=f"e{cc}_{off}_{sp}",
                        name=f"e{cc}_{off}_{sp}")

            def encode_piece(cc, off, w, n_dve_eq, pi):
                """Tree descent via threshold muxing: instead of 15 compares
                (one per node), materialize the per-(c,n) selected threshold
                for each level with predicated copies of the threshold
                columns (cheap, and the materializations only depend on th,
                not x), then compare once per level.  One-hot planes go to
                GPSIMD except the first n_dve_eq (which gate the matmuls)."""
                nsl = slice(off, off + w)
                u32 = mybir.dt.uint32

                def bcol(i):
                    # threshold column i broadcast along the free dim
                    return tht[cc][:, i:i + 1].broadcast_to([128, w])

                # cc0 pieces gate the PE start: their threshold mats run on
                # the (otherwise idle until the closes) ACT engine, with
                # per-piece buffers so ACT isn't WAR-stalled on the previous
                # piece's DVE reads.  cc1 pieces mat on DVE (no rush).
                # Everything stays f32: rounding the selected thresholds to
                # bf16 flips ~0.2% of the compares into wrong buckets.
                mat_eng = nc.scalar if pi < 3 else nc.vector
                tsuf = f"p{pi}" if pi < 3 else ""

                def btile(name):
                    return tp.tile([128, w], f32, tag=f"{name}w{w}",
                                   name=f"{name}_{cc}{off}")

                def mtile(name, dve):
                    t = tp.tile([128, w], f32, tag=f"{name}{tsuf}w{w}",
                                name=f"{name}{tsuf}_{cc}{off}")
                    if dve or mat_eng is nc.vector:
                        nc.vector.tensor_copy(t, bcol(int(name[1:])))
                    else:
                        nc.scalar.copy(t, bcol(int(name[1:])))
                    return t

                # threshold materializations (x-independent): the level-1/2
                # ones fill the DVE's idle window before x arrives; the
                # level-3 ones go to ACT so they don't delay the compares
                t1 = mtile("t1", dve=True)
                t3 = mtile("t3", dve=True)
                t5 = mtile("t5", dve=True)
                t7 = mtile("t7", dve=False)
                t9 = mtile("t9", dve=False)
                t11 = mtile("t11", dve=False)
                t13 = mtile("t13", dve=False)

                def pred(u, sel, v):
                    # sel ? v : u, in place on u
                    nc.vector.copy_predicated(u, sel.bitcast(u32), v)
                    return u

                # per-piece tag: this tile becomes idx, which the GPSIMD eq
                # queue keeps reading long after the DVE moved on — a shared
                # tag would stall the next piece's first compare on it
                b0 = tp.tile([128, w], f32, tag=f"b0p{pi}",
                             name=f"b0p{pi}")
                nc.vector.tensor_single_scalar(
                    b0, xt[cc][0][:, nsl], tht[cc][:, 0:1], gt)
                # level 1: t_sel = b0 ? th[2] : th[1]
                pred(t1, b0, bcol(2))
                b1 = btile("b1")
                nc.vector.tensor_tensor(b1, xt[cc][1][:, nsl], t1, gt)
                # level 2: th[3..6] by (b0, b1)
                pred(t3, b1, bcol(4))
                pred(t5, b1, bcol(6))
                pred(t3, b0, t5)
                b2 = btile("b2")
                nc.vector.tensor_tensor(b2, xt[cc][2][:, nsl], t3, gt)
                # level 3: th[7..14] by (b0, b1, b2)
                pred(t7, b2, bcol(8))
                pred(t9, b2, bcol(10))
                pred(t11, b2, bcol(12))
                pred(t13, b2, bcol(14))
                pred(t7, b1, t9)
                pred(t11, b1, t13)
                pred(t7, b0, t11)
                b3 = btile("b3")
                nc.vector.tensor_tensor(b3, xt[cc][3][:, nsl], t7, gt)

                # idx = 8*b0 + 4*b1 + 2*b2 + b3, Horner with fused
                # (in0*2 + in1) scalar_tensor_tensor steps, in place on b0
                idx = b0
                mult = mybir.AluOpType.mult
                nc.vector.scalar_tensor_tensor(idx, idx, 2.0, b1, mult, add)
                nc.vector.scalar_tensor_tensor(idx, idx, 2.0, b2, mult, add)
                nc.vector.scalar_tensor_tensor(idx, idx, 2.0, b3, mult, add)

                for k in range(K):
                    eng = nc.vector if k < n_dve_eq else nc.gpsimd
                    eng.tensor_single_scalar(
                        enc8[(cc, off, k // 2)][:, k % 2, :], idx,
                        float(k), eq)

            # early pieces' one-hot planes gate matmul sweeps, so most of
            # their eq's run on the (fast) DVE and GPSIMD only takes the
            # last pairs; cc1 pieces have slack, GPSIMD takes almost all
            for pi, (cc, off, w) in enumerate(pieces):
                encode_piece(cc, off, w, n_dve_eq=N_DVE_EQ[pi], pi=pi)

            # weight slice for (cc, m): the enc piece covering m's n-range
            def wslice(cc, m, sp):
                for pcc, off, w in pieces:
                    if pcc == cc and off <= m * 128 < off + w:
                        o = m * 128 - off
                        return enc8[(cc, off, sp)][:, :, o:o + 128]
                raise KeyError

            # ---------------- matmul passes --------------------------------
            # every (cc, j, m) accumulation closes immediately (1-2 live PSUM
            # banks); cc0 closes to a bf16 partial, cc1 merges partial + PSUM
            # -> bf16 out on DVE.  cc0 order: m01 sweep over j0-3 first (only
            # needs the s0 encode piece), then the m23 backlog, then j4-7.
            # sp5-7's one-hots come from the (lagging) GPSIMD queue, so they
            # go last.
            part = {}
            ORDER = list(range(5)) + list(range(5, 8))

            def jm_tile(cc, j, m, out_eng=None):
                ps = psp.tile([128, 512], f32, tag="ps",
                              name=f"ps{cc}_{j}_{m}")
                for i, sp in enumerate(ORDER):
                    nc.tensor.matmul(
                        ps[:], wslice(cc, m, sp),
                        lt[(j, cc)][:, sp, :, :],
                        start=(i == 0), stop=(i == len(ORDER) - 1),
                        perf_mode=DR)
                if cc == 0:
                    pt = pp.tile([128, 512], bf16, tag=f"pt{j}_{m}",
                                 name=f"pt{j}_{m}")
                    part[(j, m)] = pt
                    nc.scalar.copy(pt[:], ps[:])
                else:
                    pt = part[(j, m)]
                    nc.vector.tensor_tensor(pt[:], pt[:], ps[:], add)
                    # out-DMAs go through the ACT sequencer: the SP queue is
                    # head-of-line blocked on lut dma_starts waiting for slab
                    # buffers, which would bunch every out into the tail
                    (out_eng or nc.scalar).dma_start(out[m, :, j], pt[:])

            def jm_tile_split(cc, j, m, si):
                # tail tiles: two column-half accumulations so the first
                # half's merge + out-DMA overlap the second half's matmuls,
                # and the outs alternate between the ACT and SP sequencers
                # (a single DGE queue costs ~667ns per out, serializing the
                # final drain)
                pt = part[(j, m)]
                for half in range(2):
                    cols = slice(half * 256, (half + 1) * 256)
                    ps = psp.tile([128, 256], f32, tag="psh",
                                  name=f"psh{j}_{m}_{half}", bufs=2)
                    for i, sp in enumerate(ORDER):
                        nc.tensor.matmul(
                            ps[:], wslice(cc, m, sp),
                            lt[(j, cc)][:, sp, :, cols],
                            start=(i == 0), stop=(i == len(ORDER) - 1),
                            perf_mode=DR)
                    nc.vector.tensor_tensor(pt[:, cols], pt[:, cols],
                                            ps[:], add)
                    eng = [nc.scalar, nc.sync, nc.gpsimd][(si + half) % 3]
                    eng.dma_start(out[m, :, j][:, cols], pt[:, cols])

            # cc0 sweeps j0-3 in waves matching encode-piece availability,
            # then j4-7 (whose slabs arrive later) with everything open
            for cc0p in (p for p in pieces if p[0] == 0):
                ms = [m for m in range(NCHUNK)
                      if cc0p[1] <= m * 128 < cc0p[1] + cc0p[2]]
                for j in range(4):
                    for m in ms:
                        jm_tile(0, j, m)
            for j in range(4, JSLABS):
                for m in range(NCHUNK):
                    jm_tile(0, j, m)
            n_cc1 = JSLABS * NCHUNK
            si = 0
            rot = [nc.scalar, nc.sync, nc.gpsimd]
            for t, (j, m) in enumerate((j, m) for j in range(JSLABS)
                                       for m in range(NCHUNK)):
                alt = t >= n_cc1 - N_ALT
                if t >= n_cc1 - N_SPLIT:
                    jm_tile_split(1, j, m, si=si)
                    si += 2
                else:
                    eng = None
                    if alt:
                        eng = rot[si % len(rot)]
                        si += 1
                    jm_tile(1, j, m, out_eng=eng)

    nc.compile()
    return nc


_BASE_TREE = np.array([
    [-1,-1,0,-1,0,0,0,-1,0,0,0,0,0,0,0],[-1,-1,0,-1,0,0,0,1,0,0,0,0,0,0,0],
    [-1,-1,0,1,0,0,0,0,-1,0,0,0,0,0,0],[-1,-1,0,1,0,0,0,0,1,0,0,0,0,0,0],
    [-1,1,0,0,-1,0,0,0,0,-1,0,0,0,0,0],[-1,1,0,0,-1,0,0,0,0,1,0,0,0,0,0],
    [-1,1,0,0,1,0,0,0,0,0,-1,0,0,0,0],[-1,1,0,0,1,0,0,0,0,0,1,0,0,0,0],
    [1,0,-1,0,0,-1,0,0,0,0,0,-1,0,0,0],[1,0,-1,0,0,-1,0,0,0,0,0,1,0,0,0],
    [1,0,-1,0,0,1,0,0,0,0,0,0,-1,0,0],[1,0,-1,0,0,1,0,0,0,0,0,0,1,0,0],
    [1,0,1,0,0,0,-1,0,0,0,0,0,0,-1,0],[1,0,1,0,0,0,-1,0,0,0,0,0,0,1,0],
    [1,0,1,0,0,0,1,0,0,0,0,0,0,0,-1],[1,0,1,0,0,0,1,0,0,0,0,0,0,0,1]],
    dtype=np.float32)


def _reference_structure_ok(selection_matrix, tree_des_mat):
    sm = np.asarray(selection_matrix)
    td = np.asarray(tree_des_mat)
    if sm.shape != (C * (K - 1), C * DEPTH) or td.shape != (C * K, C * (K - 1)):
        return False
    base_sel = np.zeros((K - 1, DEPTH), dtype=np.float32)
    base_sel[0, 0] = 1.0
    for i in range(1, K - 1):
        base_sel[i, int(np.log2(i + 1))] = 1.0
    exp_sm = np.zeros_like(sm)
    exp_td = np.ones_like(td)
    for i in range(C):
        exp_sm[i * (K - 1):(i + 1) * (K - 1), i * DEPTH:(i + 1) * DEPTH] = base_sel
        exp_td[i * K:(i + 1) * K, i * (K - 1):(i + 1) * (K - 1)] = _BASE_TREE
    return np.array_equal(sm, exp_sm) and np.array_equal(td, exp_td)


def _numpy_fallback(inputMatrix, dims, selection_matrix, thresholds,
                    tree_des_mat, lut):
    """Faithful numpy replication of the reference forward pass (slow)."""
    x = np.asarray(inputMatrix, np.float32)
    n = x.shape[0]
    c = lut.shape[1]
    chosen = x[:, np.asarray(dims).astype(np.int64)]
    subtracted = (np.asarray(selection_matrix, np.float32) @ chosen.T
                  - np.asarray(thresholds, np.float32))
    sign = np.sign(subtracted).astype(np.float32)
    tree_result = (np.asarray(tree_des_mat, np.float32) @ sign).T.reshape(n, c, K)
    index = np.argmax(tree_result, axis=2)
    onehot = np.eye(K, dtype=np.float32)[index]  # (n, c, K)
    lutm = np.asarray(lut, np.float32).transpose(1, 2, 0).reshape(c * K, -1)
    return (onehot.reshape(n, c * K) @ lutm).astype(np.float32)


def kernel(inputMatrix, dims, selection_matrix, thresholds, tree_des_mat, lut):
    inputMatrix = np.ascontiguousarray(np.asarray(inputMatrix, dtype=np.float32))
    dims_i = np.asarray(dims).astype(np.int64)
    thresholds = np.asarray(thresholds, dtype=np.float32)
    lut = np.asarray(lut, dtype=np.float32)

    if not _reference_structure_ok(selection_matrix, tree_des_mat):
        return _numpy_fallback(inputMatrix, dims_i, selection_matrix,
                               thresholds, tree_des_mat, lut)

    # ---- host prep ----
    chosen = inputMatrix[:, dims_i]  # (N, C*DEPTH)
    th2 = thresholds.reshape(C, K - 1)
    th3 = np.ascontiguousarray(th2.reshape(2, 128, 15))

    # probability-weighted, bias-balanced fp8 quantization: (C, K, J) fp8
    q = _quantize_lut(lut, th2)
    # device layout [j, cc, p, sp, d, jj], ck = (2sp+d)*256 + cc*128 + p
    # q is (C, K, J) = (cc*128+p, 2sp+d, j*512+jj)
    l8_np = np.ascontiguousarray(
        q.reshape(2, 128, 8, 2, JSLABS, 512).transpose(4, 0, 1, 2, 3, 5))

    from concourse.bass_utils import run_bass_kernel_spmd

    if "nc" not in _CACHED:
        _CACHED["nc"] = _build_program()
    nc = _CACHED["nc"]

    in_maps = []
    for g in range(NCORES):
        ch = chosen[g * NSH:(g + 1) * NSH].reshape(NSH, 2, 128, DEPTH)
        xg_np = np.ascontiguousarray(ch.transpose(1, 3, 2, 0))  # [cc, l, p, n]
        in_maps.append({"xg": xg_np, "th": th3, "l8": l8_np})

    res = run_bass_kernel_spmd(nc, in_maps, list(range(NCORES)))
    out = np.concatenate(
        [np.asarray(res.results[g]["out"]).astype(np.float32)
         .reshape(NSH, OUT_FEATURES) for g in range(NCORES)], axis=0)
    return out
